# revision 2
# baseline (speedup 1.0000x reference)
"""Trainium2 Bass kernel for the 4-layer spiking autoencoder (data parallel, 8 cores).

Math per layer (uniform bin edges, verified against jnp.digitize semantics):
    spikes = digitize(x, bins) - 1 ;  vals = max(spikes,0)*h  (h = bins[1]-bins[0])
          == clip(floor((x - bins[0]) / h), 0, 255) * h
    out = clip(vals @ W.T + b, 0, 1000)

Device mapping:
  - quantize: ACT activation  u8 <- (x*inv_h + bias)  (RNE cast with -0.5 bias
    == floor; u8 write saturates to [0,255] == the clip)
  - the inter-layer clip(.,0,1000) is fully absorbed by the next quantize's
    saturation (x<0 -> 0; x>20 -> 255 either way)
  - h is folded into the (transposed) weights host-side; matmuls run in bf16
    (quantized activations are integers <=255: exact in bf16) with f32 PSUM
  - final layer: DVE tensor_scalar clamp [0,1000] from PSUM (plus an ACT bias
    pass if b4 != 0), written as f32
Layout: batch on the moving/free dim. Host pre-transposes features to
[784, B_shard] per core and transposes the [784, B_shard] result back.
"""
import sys

if "/opt/trn_rl_repo" not in sys.path:
    sys.path.insert(0, "/opt/trn_rl_repo")

import numpy as np
import ml_dtypes

import concourse.bass as bass
import concourse.tile as tile
from concourse import mybir
from concourse.bass_utils import run_bass_kernel_spmd

B = 65536
D = 784          # in/out dim
H = 128          # hidden
NCORES = 8
BS = B // NCORES  # 8192 batch rows per core
T = 512           # batch tile (moving free dim / PSUM bank)
KC = 112          # contraction chunk for the 784 dims (7 x 112)
NCH = D // KC     # 7

F32 = mybir.dt.float32
BF16 = mybir.dt.bfloat16
U8 = mybir.dt.uint8


def _fix_multiwait(nc):
    """walrus here allows only ONE sync wait per instruction; split extras
    onto same-engine NoOps placed immediately before the instruction."""
    import concourse.mybir as mb
    ctr = 0
    for f in nc.m.functions:
        for blk in f.blocks:
            il = blk.instructions
            newl = []
            changed = False
            for inst in il:
                si = getattr(inst, "sync_info", None)
                ow = list(si.on_wait) if si is not None and si.on_wait else []
                if len(ow) > 1:
                    for w in ow[:-1]:
                        nop = mb.InstNoOp(name=f"waitsplit-{ctr}", ins=[], outs=[])
                        ctr += 1
                        nop.engine = inst.engine
                        nop.sync_info = mb.SyncInfo(on_wait=[w], on_update=[])
                        nop.debug = inst.debug
                        newl.append(nop)
                    si.on_wait = [ow[-1]]
                    inst.sync_info = si
                    changed = True
                newl.append(inst)
            if changed:
                il.clear()
                il.extend(newl)


def _build(nc, scales, has_b4):
    """scales: dict with inv_h0..3, q0_bias (floats). Biases via small inputs."""
    xT = nc.declare_dram_parameter("xT", [D, BS], F32, isOutput=False)
    # bf16 multi-term weight splits: w = sum(terms); exact fp32 reconstruction
    w1 = [nc.declare_dram_parameter(f"w1{s}", [D, H], BF16, isOutput=False)
          for s in range(3)]                                             # (h0*W1).T
    w2 = [nc.declare_dram_parameter(f"w2{s}", [H, H], BF16, isOutput=False)
          for s in range(3)]                                             # (h1*W2).T
    w3 = [nc.declare_dram_parameter(f"w3{s}", [H, H], BF16, isOutput=False)
          for s in range(3)]                                             # (h2*W3).T
    w4 = nc.declare_dram_parameter("w4", [H, D], BF16, isOutput=False)   # (h3*W4).T
    qb1 = nc.declare_dram_parameter("qb1", [H], F32, isOutput=False)
    qb2 = nc.declare_dram_parameter("qb2", [H], F32, isOutput=False)
    qb3 = nc.declare_dram_parameter("qb3", [H], F32, isOutput=False)
    if has_b4:
        b4p = nc.declare_dram_parameter("b4p", [D], F32, isOutput=False)
    outT = nc.declare_dram_parameter("outT", [D, BS], F32, isOutput=True)

    with tile.TileContext(nc) as tc:
        with (
            tc.tile_pool(name="wp", bufs=1) as wp,
            tc.tile_pool(name="xp", bufs=3) as xp,
            tc.tile_pool(name="q8a", bufs=2) as q8a,
            tc.tile_pool(name="qba", bufs=2) as qba,
            tc.tile_pool(name="q8b", bufs=2) as q8b,
            tc.tile_pool(name="qbb", bufs=2) as qbb,
            tc.tile_pool(name="stp", bufs=3) as stp,
            tc.tile_pool(name="ps1", bufs=2, space="PSUM") as ps1p,
            tc.tile_pool(name="ps2", bufs=2, space="PSUM") as ps2p,
            tc.tile_pool(name="ps3", bufs=2, space="PSUM") as ps3p,
            tc.tile_pool(name="ps4", bufs=2, space="PSUM") as ps4p,
        ):
            # ---- constants (loaded once) ----
            w1t, w2t, w3t = [], [], []
            for s in range(3):
                t1 = wp.tile([KC, NCH * H], BF16, tag=f"w1t{s}")
                nc.gpsimd.dma_start(
                    t1[:].rearrange("k (c m) -> k c m", c=NCH),
                    w1[s][:].rearrange("(c k) m -> k c m", k=KC),
                )
                w1t.append(t1)
                t2 = wp.tile([H, H], BF16, tag=f"w2t{s}")
                nc.gpsimd.dma_start(t2[:], w2[s][:])
                w2t.append(t2)
                t3 = wp.tile([H, H], BF16, tag=f"w3t{s}")
                nc.gpsimd.dma_start(t3[:], w3[s][:])
                w3t.append(t3)
            w4t = wp.tile([H, D], BF16)
            nc.gpsimd.dma_start(w4t[:], w4[:])
            qb1t = wp.tile([H, 1], F32)
            nc.gpsimd.dma_start(qb1t[:], qb1[:].rearrange("(m o) -> m o", o=1))
            qb2t = wp.tile([H, 1], F32)
            nc.gpsimd.dma_start(qb2t[:], qb2[:].rearrange("(m o) -> m o", o=1))
            qb3t = wp.tile([H, 1], F32)
            nc.gpsimd.dma_start(qb3t[:], qb3[:].rearrange("(m o) -> m o", o=1))
            if has_b4:
                b4t = wp.tile([KC, NCH], F32)
                nc.gpsimd.dma_start(
                    b4t[:].rearrange("k (c o) -> k c o", o=1),
                    b4p[:].rearrange("(c k o) -> k c o", k=KC, o=1),
                )

            ID = mybir.ActivationFunctionType.Identity
            CP = mybir.ActivationFunctionType.Copy
            MAX = mybir.AluOpType.max
            MIN = mybir.AluOpType.min

            for t in range(BS // T):
                ts = slice(t * T, (t + 1) * T)
                # ---- load x tile [112, 7*T] (7 chunks of 112 dims) ----
                xt = xp.tile([KC, NCH * T], F32)
                nc.gpsimd.dma_start(
                    xt[:].rearrange("k (c b) -> k c b", c=NCH),
                    xT[:, ts].rearrange("(c k) b -> k c b", k=KC),
                )
                # ---- L0 quantize: u8 <- floor(x*inv_h0 + q0_bias) sat ----
                q80 = q8a.tile([KC, NCH * T], U8)
                nc.scalar.activation(q80[:], xt[:], CP,
                                     bias=scales["q0_bias"], scale=scales["inv_h0"])
                qb0 = qba.tile([KC, NCH * T], BF16)
                nc.vector.tensor_copy(qb0[:], q80[:])

                # ---- L1: psum = sum_s sum_c w1[s]_c.T @ q0_c ----
                ps1 = ps1p.tile([H, T], F32)
                for s in range(3):
                    for c in range(NCH):
                        nc.tensor.matmul(ps1[:], w1t[s][:, c * H:(c + 1) * H],
                                         qb0[:, c * T:(c + 1) * T],
                                         start=(s == 0 and c == 0),
                                         stop=(s == 2 and c == NCH - 1))
                q81 = q8b.tile([H, T], U8)
                nc.scalar.activation(q81[:], ps1[:], ID,
                                     bias=qb1t[:, 0:1], scale=scales["inv_h1"])
                qb1_ = qbb.tile([H, T], BF16)
                nc.vector.tensor_copy(qb1_[:], q81[:])

                # ---- L2 ----
                ps2 = ps2p.tile([H, T], F32)
                for s in range(3):
                    nc.tensor.matmul(ps2[:], w2t[s][:], qb1_[:],
                                     start=(s == 0), stop=(s == 2))
                q82 = q8b.tile([H, T], U8)
                nc.scalar.activation(q82[:], ps2[:], ID,
                                     bias=qb2t[:, 0:1], scale=scales["inv_h2"])
                qb2_ = qbb.tile([H, T], BF16)
                nc.vector.tensor_copy(qb2_[:], q82[:])

                # ---- L3 ----
                ps3 = ps3p.tile([H, T], F32)
                for s in range(3):
                    nc.tensor.matmul(ps3[:], w3t[s][:], qb2_[:],
                                     start=(s == 0), stop=(s == 2))
                q83 = q8b.tile([H, T], U8)
                nc.scalar.activation(q83[:], ps3[:], ID,
                                     bias=qb3t[:, 0:1], scale=scales["inv_h3"])
                qb3_ = qbb.tile([H, T], BF16)
                nc.vector.tensor_copy(qb3_[:], q83[:])

                # ---- L4: 7 output chunks + clamp [0,1000] ----
                st = stp.tile([KC, NCH * T], F32)
                for c in range(NCH):
                    ps4 = ps4p.tile([KC, T], F32)
                    nc.tensor.matmul(ps4[:], w4t[:, c * KC:(c + 1) * KC], qb3_[:],
                                     start=True, stop=True)
                    if has_b4:
                        zt = stp.tile([KC, T], F32, tag="zb4")
                        nc.scalar.activation(zt[:], ps4[:], ID,
                                             bias=b4t[:, c:c + 1], scale=1.0)
                        nc.vector.tensor_scalar(st[:, c * T:(c + 1) * T], zt[:],
                                                0.0, 1000.0, MAX, MIN)
                    else:
                        nc.vector.tensor_scalar(st[:, c * T:(c + 1) * T], ps4[:],
                                                0.0, 1000.0, MAX, MIN)
                # ---- store out tile ----
                nc.gpsimd.dma_start(
                    outT[:, ts].rearrange("(c k) b -> k c b", k=KC),
                    st[:].rearrange("k (c b) -> k c b", c=NCH),
                )
    _fix_multiwait(nc)
    return nc


def _prep(inputs):
    """Host-side: scales, scaled+transposed weights, per-core shards."""
    f64 = np.float64
    bins = [inputs["bins0"], inputs["bins1"], inputs["bins2"], inputs["bins3"]]
    h = [f64(b[1]) - f64(b[0]) for b in bins]
    lo = [f64(b[0]) for b in bins]
    inv_h = [1.0 / hi for hi in h]
    scales = {
        "inv_h0": float(inv_h[0]),
        "inv_h1": float(inv_h[1]),
        "inv_h2": float(inv_h[2]),
        "inv_h3": float(inv_h[3]),
        # bias for L0 quantize: -lo*inv_h - 0.5 (RNE(y-0.5)=floor(y))
        "q0_bias": float(-lo[0] * inv_h[0] - 0.5),
    }
    W1, W2, W3, W4 = inputs["W1"], inputs["W2"], inputs["W3"], inputs["W4"]
    b1, b2, b3, b4 = inputs["b1"], inputs["b2"], inputs["b3"], inputs["b4"]
    bf = ml_dtypes.bfloat16

    def split_terms(w, n):
        """split f32 array into n bf16 terms summing (near-)exactly to w"""
        terms = []
        r = w.astype(np.float32)
        for _ in range(n):
            t = r.astype(bf)
            terms.append(np.ascontiguousarray(t))
            r = r - t.astype(np.float32)
        return terms

    w1s = split_terms((W1.astype(f64) * h[0]).T.astype(np.float32), 3)    # [784,128]
    w2s = split_terms((W2.astype(f64) * h[1]).T.astype(np.float32), 3)    # [128,128]
    w3s = split_terms((W3.astype(f64) * h[2]).T.astype(np.float32), 3)    # [128,128]
    w4 = np.ascontiguousarray((W4.astype(f64) * h[3]).T.astype(np.float32).astype(bf))  # [128,784]
    # quantize-bias vectors for L1..L3 quantize stages: (b_i - lo_i)*inv_h_i - 0.5
    qb1 = ((b1.astype(f64) - lo[1]) * inv_h[1] - 0.5).astype(np.float32)
    qb2 = ((b2.astype(f64) - lo[2]) * inv_h[2] - 0.5).astype(np.float32)
    qb3 = ((b3.astype(f64) - lo[3]) * inv_h[3] - 0.5).astype(np.float32)
    has_b4 = bool(np.any(b4 != 0))
    consts = {"w4": w4, "qb1": qb1, "qb2": qb2, "qb3": qb3}
    for s in range(3):
        consts[f"w1{s}"] = w1s[s]
        consts[f"w2{s}"] = w2s[s]
        consts[f"w3{s}"] = w3s[s]
    if has_b4:
        consts["b4p"] = b4.astype(np.float32)
    return scales, consts, has_b4


def _run(inputs, trace=False, **run_kwargs):
    scales, consts, has_b4 = _prep(inputs)
    nc = bass.Bass()
    _build(nc, scales, has_b4)

    features = inputs["features"]
    assert features.shape == (B, D), features.shape
    in_maps = []
    for i in range(NCORES):
        shard = features[i * BS:(i + 1) * BS]
        m = dict(consts)
        m["xT"] = np.ascontiguousarray(shard.T.astype(np.float32, copy=False))
        in_maps.append(m)

    res = run_bass_kernel_spmd(nc, in_maps, core_ids=list(range(NCORES)),
                               trace=trace, **run_kwargs)
    out = np.empty((B, D), np.float32)
    for i in range(NCORES):
        out[i * BS:(i + 1) * BS] = res.results[i]["outT"].T
    return out, res


def kernel(**inputs):
    out, _ = _run(inputs)
    return out
